# revision 2
# baseline (speedup 1.0000x reference)
"""Trainium2 Bass kernel for nn_CompletenessLoss (OHEM hinge loss with top-k).

Self-contained: accepts FULL inputs, shards over 8 NeuronCores internally
(data-parallel over the group dimension), returns the full scalar output.

Math (reference):
  scores[i]  = pred[i, labels[i]-1]
  groups of 64 rows: first 8 are "positive", last 56 are "negative"
  pos_ls = sum over all positive rows of relu(1 - s)
  neg_ls = sum over groups of (sum of top-9 of relu(1 + s) over 56 negatives)
  out    = (pos_ls + neg_ls) / (num_pos + int(num_neg * 0.17))

V3 strategy (per core, 32768 rows = 128 partitions x 256 rows):
  ALL label gathers run on GPSIMD ap_gather (measured ~2 cyc/idx), fed by a
  16-row-per-partition DMA pipeline of pred in bf16.  ap_gather shares one
  index list per 16-partition core, so the host packs position 16j+q with
  the index of partition (16g+q)'s row j; each partition's wanted PAIR
  (d=2, 32-bit blocks) lands on the "diagonal" slot 16j+(p%16).
  Extraction per 64-row block on DVE: multiply by a static per-partition
  Q-mask (zeroes the 15 wrong slots, keeps the pair), 4 pairwise folds
  (exact: one nonzero pair among zeros), a parity mask picks the even/odd
  element, and a 2-wide reduce writes f32 scores.  Phase 2 (hinge + top-9
  via max8/match_replace) runs per group right behind its block.
  HBM traffic/core: 13.1 MB pred + ~0.7 MB metadata (vs 23 MB in V2).
"""

import numpy as np

# Problem geometry (hardcoded per the harness contract).
N_FULL = 262144
D = 200                      # pred_dim
GS = 64                      # sample_group_size
SS = 8                       # sample_split (positives per group)
OHEM_RATIO = 0.17
KEEP = int((GS - SS) * OHEM_RATIO)   # 9 hardest negatives kept per group

N_CORES = 8
ROWS = N_FULL // N_CORES     # 32768 rows per core
P = 128                      # SBUF partitions
NTILES = ROWS // P           # 256 rows per partition = 4 groups
CHUNK = 16                   # rows-per-partition per DMA/gather chunk
BLK = 64                     # rows-per-partition per extraction block (=GS)
HD = D // 2                  # 100 pair-blocks per row

_compiled = None             # cached program so repeat calls skip rebuild


def build_nc():
    """Build the per-core Bass program (SPMD across the 8 cores)."""
    import concourse.bacc as bacc
    import concourse.tile as tile
    from concourse import mybir

    f32 = mybir.dt.float32
    bf16 = mybir.dt.bfloat16
    i16 = mybir.dt.int16

    nc = bacc.Bacc("TRN2", target_bir_lowering=False, debug=False,
                   num_devices=N_CORES)
    pred_t = nc.dram_tensor("pred", [ROWS, D], bf16, kind="ExternalInput")
    # idx[p, t] = (t%CHUNK)*HD + (lab[p,t]-1)>>1, int16 gather block index
    idx_t = nc.dram_tensor("idx", [P, NTILES], i16, kind="ExternalInput")
    # par[p, 2t+e] = (e == (lab-1)&1), bf16 parity-pair mask
    par_t = nc.dram_tensor("par", [P, NTILES * 2], bf16, kind="ExternalInput")
    # qrep[p, 32j + 2q+e] = (q == p%16), j < BLK: static slot mask
    qrep_t = nc.dram_tensor("qrep", [P, BLK * 32], bf16, kind="ExternalInput")
    out_t = nc.dram_tensor("partial", [P, 2], f32, kind="ExternalOutput")

    with tile.TileContext(nc) as tc:
        _body(tc, pred_t.ap(), idx_t.ap(), par_t.ap(), qrep_t.ap(),
              out_t.ap())
    nc.compile()
    return nc


def _body(tc, pred, idx, par, qrep, out):
    from concourse import mybir
    from contextlib import ExitStack

    nc = tc.nc
    f32 = mybir.dt.float32
    bf16 = mybir.dt.bfloat16
    i16 = mybir.dt.int16
    AX = mybir.AxisListType
    OP = mybir.AluOpType
    AF = mybir.ActivationFunctionType

    with ExitStack() as ctx:
        singles = ctx.enter_context(tc.tile_pool(name="singles", bufs=1))
        ph2 = ctx.enter_context(tc.tile_pool(name="ph2", bufs=2))

        # --- one-time inputs (small, on the scalar queue) ---
        idxs = singles.tile([P, NTILES], i16)
        nc.scalar.dma_start(out=idxs, in_=idx)
        pars = singles.tile([P, NTILES, 2], bf16)
        nc.scalar.dma_start(
            out=pars.rearrange("p t e -> p (t e)"), in_=par)
        qmask = singles.tile([P, BLK * 32], bf16)
        nc.scalar.dma_start(out=qmask, in_=qrep)

        pred_sb = singles.tile([P, NTILES, D], bf16)
        out2 = singles.tile([P, NTILES * 16, 2], bf16)
        scores = singles.tile([P, NTILES], f32)

        # --- warm-up: pay one-time engine costs before the first chunk ---
        # GPSIMD: ap_gather ucode IRAM load (~6us) via a tiny dummy gather.
        wz_idx = singles.tile([P, 16], i16)
        nc.gpsimd.memset(wz_idx, 0)
        wdat = singles.tile([P, 16, 2], bf16)
        nc.gpsimd.memset(wdat.rearrange("p a b -> p (a b)"), 0)
        warm3 = singles.tile([P, 16, 2], bf16)
        nc.gpsimd.ap_gather(out_ap=warm3, in_ap=wdat,
                            idxs_ap=wz_idx[:, 0:1],
                            channels=P, num_elems=16, d=2, num_idxs=16)
        # DVE: first-op dispatch warm.
        wv = singles.tile([P, 2], f32)
        nc.vector.memset(wv, 0.0)
        nc.vector.tensor_scalar(out=wv, in0=wv, scalar1=0.0, scalar2=1.0,
                                op0=OP.mult, op1=OP.mult)
        # Scalar: Relu activation-table load.
        wa = singles.tile([P, 2], f32)
        nc.scalar.activation(out=wa, in_=wv, func=AF.Relu,
                             bias=1.0, scale=-1.0)

        # --- accumulators for phase 2 ---
        gpp = NTILES // GS
        pp = singles.tile([P, gpp], f32)             # pos sums per group
        negacc = singles.tile([P, 2 * gpp], f32)     # top8-sum & 9th cols

        # --- pipeline: DMA chunk -> gather chunk -> (per BLK) extract+phase2
        pred_v = pred.rearrange("(p t) j -> p t j", p=P)
        nchunks = NTILES // CHUNK
        for ci in range(nchunks):
            tb = ci * CHUNK
            nc.sync.dma_start(out=pred_sb[:, tb:tb + CHUNK, :],
                              in_=pred_v[:, tb:tb + CHUNK, :])
            nc.gpsimd.ap_gather(
                out_ap=out2[:, tb * 16:(tb + CHUNK) * 16, :],
                in_ap=pred_sb[:, tb:tb + CHUNK, :].rearrange(
                    "p t (a b) -> p (t a) b", b=2),
                idxs_ap=idxs[:, tb:tb + CHUNK],
                channels=P, num_elems=CHUNK * HD, d=2,
                num_idxs=CHUNK * 16)

            if (tb + CHUNK) % BLK != 0:
                continue

            # --- extraction for the finished 64-row block ---
            bb = (tb + CHUNK) - BLK          # block start row
            g = bb // GS                     # group index (BLK == GS)
            o2 = out2[:, bb * 16:(bb + BLK) * 16, :]
            nc.vector.tensor_tensor(
                out=o2.rearrange("p a b -> p (a b)"),
                in0=o2.rearrange("p a b -> p (a b)"),
                in1=qmask, op=OP.mult)
            o3 = o2.rearrange("p (t a) b -> p t (a b)", a=16)  # [P, 64, 32]
            for half in (16, 8, 4, 2):
                nc.vector.tensor_tensor(
                    out=o3[:, :, 0:half], in0=o3[:, :, 0:half],
                    in1=o3[:, :, half:2 * half], op=OP.add)
            nc.vector.tensor_tensor(
                out=o3[:, :, 0:2], in0=o3[:, :, 0:2],
                in1=pars[:, bb:bb + BLK, :], op=OP.mult)
            nc.vector.tensor_reduce(
                out=scores[:, bb:bb + BLK], in_=o3[:, :, 0:2],
                axis=AX.X, op=OP.add)

            # --- phase 2 for this group: hinge + top-9 ---
            stg = scores[:, g * GS:(g + 1) * GS]
            ptmp = ph2.tile([P, SS], f32, tag="ptmp")
            nc.scalar.activation(
                out=ptmp, in_=stg[:, 0:SS], func=AF.Relu,
                bias=1.0, scale=-1.0, accum_out=pp[:, g:g + 1])
            nl = ph2.tile([P, GS - SS], f32, tag="nl")
            nc.scalar.activation(
                out=nl, in_=stg[:, SS:GS],
                func=AF.Relu, bias=1.0, scale=1.0)
            m8 = ph2.tile([P, 8], f32, tag="m8")
            nc.vector.max(out=m8, in_=nl)
            nc.vector.match_replace(
                out=nl, in_to_replace=m8, in_values=nl, imm_value=-1.0)
            nc.vector.tensor_reduce(
                out=negacc[:, 2 * g:2 * g + 1], in_=m8, axis=AX.X, op=OP.add)
            nc.vector.tensor_reduce(
                out=negacc[:, 2 * g + 1:2 * g + 2], in_=nl, axis=AX.X,
                op=OP.max)

        # --- final per-partition reduction -> [P, 2] ---
        res = singles.tile([P, 2], f32)
        nc.vector.tensor_reduce(out=res[:, 0:1], in_=pp, axis=AX.X, op=OP.add)
        nc.vector.tensor_reduce(out=res[:, 1:2], in_=negacc, axis=AX.X,
                                op=OP.add)
        nc.sync.dma_start(out=out, in_=res)


def _get_compiled():
    global _compiled
    if _compiled is None:
        _compiled = build_nc()
    return _compiled


def _prep_core_inputs(pred, labels):
    """Split full inputs into per-core input maps."""
    import ml_dtypes
    pred = np.asarray(pred).astype(ml_dtypes.bfloat16)
    lab = np.asarray(labels).astype(np.int64)
    jloc = (np.arange(NTILES, dtype=np.int64) % CHUNK)[None, :]  # [1, NT]
    # static slot mask: qrep[p, 32j + 2q+e] = (q == p%16)
    qsel = (np.arange(P, dtype=np.int64) % 16)                   # [P]
    qe = np.arange(32, dtype=np.int64) // 2                      # [32] -> q
    qrep = (qe[None, :] == qsel[:, None]).astype(ml_dtypes.bfloat16)
    qrep = np.ascontiguousarray(np.tile(qrep, (1, BLK)))         # [P, BLK*32]
    in_maps = []
    for c in range(N_CORES):
        sl = slice(c * ROWS, (c + 1) * ROWS)
        lab_sh = (lab[sl] - 1).reshape(P, NTILES)                # int64
        idxs = (jloc * HD + (lab_sh >> 1)).astype(np.int16)
        e = (lab_sh & 1)                                         # [P, NT]
        par = np.zeros((P, NTILES, 2), dtype=ml_dtypes.bfloat16)
        par[:, :, 0] = (e == 0)
        par[:, :, 1] = (e == 1)
        in_maps.append({
            "pred": np.ascontiguousarray(pred[sl]),
            "idx": np.ascontiguousarray(idxs),
            "par": np.ascontiguousarray(par.reshape(P, NTILES * 2)),
            "qrep": qrep,
        })
    return in_maps


def _finalize(results):
    pos = 0.0
    neg = 0.0
    for r in results:
        part = r["partial"].astype(np.float64)
        pos += part[:, 0].sum()
        neg += part[:, 1].sum()
    num_pos = (N_FULL // GS) * SS
    num_neg = N_FULL - num_pos
    denom = float(num_pos + int(num_neg * OHEM_RATIO))
    return np.float32((pos + neg) / denom)


def kernel(pred, labels, sample_split, sample_group_size):
    assert int(sample_split) == SS and int(sample_group_size) == GS
    from concourse.bass_utils import run_bass_kernel_spmd

    nc = _get_compiled()
    in_maps = _prep_core_inputs(pred, labels)
    res = run_bass_kernel_spmd(nc, in_maps, core_ids=list(range(N_CORES)))
    return _finalize(res.results)
